# revision 42
# baseline (speedup 1.0000x reference)
"""DGCNN (4x GCNConv + SortPool + Conv1d head) on 8 Trainium2 NeuronCores.

Data-parallel over graphs: each core owns 64 of the 512 graphs.
Per graph the GCN aggregation is computed densely in bf16:
    agg^T[f, i] = sum_j (y[j, f] * dinv[j]) * (adj[j, i] * dinv[i])
with adj the src-major dense adjacency-with-self-loops count matrix,
densified on the host from edge_index (a re-layout of the integer graph
structure); integer degree counts are likewise host re-layouts. All
floating-point math (rsqrt normalization, 4 GCN layers, SortPool top-k
selection+gather, conv/MLP head) runs on-device. Matmul operands are
bf16 (exact for the integer-valued adjacency), accumulation is fp32.
"""

import numpy as np

B = 512
M = 200
GPC = 64            # graphs per core
NPC = GPC * M       # nodes per core
NCORES = 8
K = 30
F = 97

_STATE = {}


def _apf(base, pairs):
    """AP with the partition dim of `base` and custom free [step,count] pairs."""
    import concourse.bass as bass
    return bass.AP(tensor=base.tensor, offset=base.offset,
                   ap=[list(base.ap[0])] + [list(p) for p in pairs])


def _build(skip=()):
    skip = set(skip)
    import concourse.bass as bass
    import concourse.bacc as bacc
    import concourse.mybir as mybir
    from concourse.tile import TileContext
    from concourse.masks import make_identity

    fp32 = mybir.dt.float32
    bf16 = mybir.dt.bfloat16
    AF = mybir.ActivationFunctionType
    OP = mybir.AluOpType

    nc = bacc.Bacc("TRN2", target_bir_lowering=False, debug=False,
                   num_devices=NCORES)

    xT_d = nc.dram_tensor("xT", [128, NPC], bf16, kind="ExternalInput")
    adjlo_d = nc.dram_tensor("adjlo", [128, NPC], bf16, kind="ExternalInput")
    adjhi_d = nc.dram_tensor("adjhi", [72, NPC], bf16, kind="ExternalInput")
    deg32_d = nc.dram_tensor("deg32", [32, NPC], bf16, kind="ExternalInput")
    degcol_d = nc.dram_tensor("degcol", [128, 128], bf16, kind="ExternalInput")
    w1_d = nc.dram_tensor("w1", [128, 32], bf16, kind="ExternalInput")
    w234_d = nc.dram_tensor("w234", [96, 32], bf16, kind="ExternalInput")
    bgcn_d = nc.dram_tensor("bgcn", [32, 4], fp32, kind="ExternalInput")
    cw1_d = nc.dram_tensor("cw1", [97, 16], bf16, kind="ExternalInput")
    cb1_d = nc.dram_tensor("cb1", [16, 1], fp32, kind="ExternalInput")
    cw2_d = nc.dram_tensor("cw2", [80, 32], bf16, kind="ExternalInput")
    cb2_d = nc.dram_tensor("cb2", [32, 1], fp32, kind="ExternalInput")
    lw1p_d = nc.dram_tensor("lw1p", [33, 1408], bf16, kind="ExternalInput")
    lw2_d = nc.dram_tensor("lw2", [128, 1], bf16, kind="ExternalInput")
    lb2_d = nc.dram_tensor("lb2", [1, 1], fp32, kind="ExternalInput")

    idxbuf_d = nc.dram_tensor("idxbuf", [GPC * 32], mybir.dt.int16,
                              kind="Internal")
    out_d = nc.dram_tensor("out", [1, GPC], fp32, kind="ExternalOutput")

    NCH = 8                 # prologue chunks
    CW = NPC // NCH         # 1600 cols per chunk

    with TileContext(nc) as tc:
        with tc.tile_pool(name="const", bufs=1) as cp:
            identb = cp.tile([64, 64], bf16)
            make_identity(nc, identb[:])
            w1 = cp.tile([128, 32], bf16)
            nc.sync.dma_start(w1[:], w1_d.ap())
            w234 = cp.tile([96, 32], bf16)
            nc.sync.dma_start(w234[:], w234_d.ap())
            bgcn = cp.tile([32, 4], fp32)
            nc.sync.dma_start(bgcn[:], bgcn_d.ap())
            cw1 = cp.tile([97, 16], bf16)
            cb1 = cp.tile([16, 1], fp32)
            cw2 = [cp.tile([16, 32], bf16, tag=f"cw2_{t}", name=f"cw2_{t}")
                   for t in range(5)]
            cb2 = cp.tile([32, 1], fp32)
            lw1p = cp.tile([33, 1408], bf16)
            lw2 = cp.tile([128, 1], bf16)
            lb2 = cp.tile([1, 1], fp32)

            def load_head_consts():
                # deferred: head weights aren't needed until the tail, so
                # keep their DMAs out of the prologue's sync queue
                nc.sync.dma_start(cw1[:], cw1_d.ap())
                nc.sync.dma_start(cb1[:], cb1_d.ap())
                for t in range(5):
                    nc.sync.dma_start(cw2[t][:],
                                      cw2_d.ap()[16 * t:16 * t + 16, :])
                nc.sync.dma_start(cb2[:], cb2_d.ap())
                nc.sync.dma_start(lw1p[:], lw1p_d.ap())
                nc.sync.dma_start(lw2[:], lw2_d.ap())
                nc.sync.dma_start(lb2[:], lb2_d.ap())

            # per-node 1/sqrt(deg) columns: col 2g = nodes 0:128 of graph g,
            # col 2g+1 = nodes 128:200 (on partitions 0:72)
            dinv_col = cp.tile([128, 128], fp32)
            dcol_s = cp.tile([128, 128], bf16, name="dcol_s")
            nc.sync.dma_start(dcol_s[:], degcol_d.ap())
            nc.scalar.activation(dinv_col[:], dcol_s[:],
                                 AF.Abs_reciprocal_sqrt)

            # hcat rows: 0:32 h1, 32:64 h2, 64:96 h3, 96 h4; col = 256*g + i
            hcat = cp.tile([112, 200 * GPC], bf16)
            topsT = cp.tile([112, 32 * GPC], fp32)
            topsB = cp.tile([112, 32 * GPC], bf16)
            c1T = cp.tile([16, 30 * GPC], bf16)

            if "agg" in skip:
                nc.gpsimd.memset(hcat[:], 0.25)

            with tc.tile_pool(name="gcn", bufs=1) as gp_pool:
                adjS_lo = gp_pool.tile([128, NPC], bf16)
                adjS_hi = gp_pool.tile([72, NPC], bf16)
                dinv_rep = gp_pool.tile([128, NPC], bf16)
                hcat32 = gp_pool.tile([112, 200 * GPC], fp32)

                with (
                    tc.tile_pool(name="stage", bufs=3) as stp,
                    tc.tile_pool(name="work", bufs=4) as wp,
                    tc.tile_pool(name="spw", bufs=2) as spw,
                    tc.tile_pool(name="xsp", bufs=4) as xsp,
                    tc.tile_pool(name="psY", bufs=4, space="PSUM") as psY,
                    tc.tile_pool(name="psG", bufs=4, space="PSUM") as psG,
                ):
                    def stage_chunk(q):
                        c0, c1 = CW * q, CW * (q + 1)
                        astage = stp.tile([128, CW], bf16, tag="astage",
                                          name="astage")
                        nc.sync.dma_start(astage[:], adjlo_d.ap()[:, c0:c1])
                        hstage = stp.tile([72, CW], bf16, tag="hstage",
                                          name="hstage")
                        nc.scalar.dma_start(hstage[:], adjhi_d.ap()[:, c0:c1])
                        return astage, hstage

                    def scale_chunk(q, tiles):
                        c0, c1 = CW * q, CW * (q + 1)
                        astage, hstage = tiles
                        nc.vector.tensor_tensor(
                            out=adjS_lo[:, c0:c1], in0=astage[:],
                            in1=dinv_rep[:, c0:c1], op=OP.mult)
                        nc.vector.tensor_tensor(
                            out=adjS_hi[:, c0:c1], in0=hstage[:],
                            in1=dinv_rep[0:72, c0:c1], op=OP.mult)

                    def pair_layer(gp, l):
                        fo = 32 if l < 3 else 1
                        py = psY.tile([128, 128], fp32, tag="y", name="py")
                        y = wp.tile([128, 128], bf16, tag="y_s", name="y")
                        for half in range(2):
                            g = 2 * gp + half
                            yo = 64 * half
                            if l == 0:
                                xs = xstash[gp // 2]
                                xo = 400 * (gp % 2) + 200 * half
                                lhs_lo = xs[:, xo:xo + 128]
                                lhs_hi = xs[:, xo + 128:xo + 200]
                                w_t = w1[:, 0:fo]
                            else:
                                r0 = 32 * (l - 1)
                                c0 = 200 * g
                                lhs_lo = hcat[r0:r0 + 32, c0:c0 + 128]
                                lhs_hi = hcat[r0:r0 + 32, c0 + 128:c0 + 200]
                                w_t = w234[r0:r0 + 32, 0:fo]
                            nc.tensor.matmul(py[:, yo:yo + fo],
                                             lhsT=lhs_lo, rhs=w_t,
                                             start=True, stop=True)
                            nc.tensor.matmul(py[0:72, yo + 32:yo + 32 + fo],
                                             lhsT=lhs_hi, rhs=w_t,
                                             start=True, stop=True)
                        # drain xw, row scale dinv[j]; blocks {lo, hi} x 2
                        nc.vector.tensor_tensor(
                            out=_apf(y[0:128, 0:1], [[64, 2], [1, fo]]),
                            in0=_apf(py[0:128, 0:1], [[64, 2], [1, fo]]),
                            in1=_apf(dinv_col[0:128, 4 * gp:4 * gp + 1],
                                     [[2, 2], [0, fo]]),
                            op=OP.mult)
                        nc.vector.tensor_tensor(
                            out=_apf(y[0:72, 32:33], [[64, 2], [1, fo]]),
                            in0=_apf(py[0:72, 32:33], [[64, 2], [1, fo]]),
                            in1=_apf(dinv_col[0:72,
                                     4 * gp + 1:4 * gp + 2],
                                     [[2, 2], [0, fo]]),
                            op=OP.mult)
                        pagg = psG.tile([32, 456], fp32, tag="agg",
                                        name="pagg")
                        for half in range(2):
                            g = 2 * gp + half
                            yo, co = 64 * half, 256 * half
                            nc.tensor.matmul(
                                pagg[0:fo, co:co + 200],
                                lhsT=y[:, yo:yo + fo],
                                rhs=adjS_lo[:, 200 * g:200 * g + 200],
                                start=True, stop=False)
                            nc.tensor.matmul(
                                pagg[0:fo, co:co + 200],
                                lhsT=y[0:72, yo + 32:yo + 32 + fo],
                                rhs=adjS_hi[:, 200 * g:200 * g + 200],
                                start=False, stop=True)
                        r0 = 32 * l if l < 3 else 96
                        nc.scalar.activation(
                            hcat[r0:r0 + fo, 400 * gp:400 * gp + 400],
                            _apf(pagg[0:fo, 0:1], [[256, 2], [1, 200]]),
                            AF.Tanh, bias=bgcn[0:fo, l:l + 1])

                    def sortpool_group(q):
                        # top-30 for graphs 8q..8q+8; cols 1600q..1600q+1600
                        s0 = 1600 * q
                        h4r = spw.tile([8, 256], bf16, tag="h4r", name="h4r")
                        nc.sync.dma_start(
                            h4r[:, 0:200],
                            _apf(hcat[96:97, s0:s0 + 1],
                                 [[200, 8], [1, 200]]))
                        nc.vector.memset(h4r[:, 200:256], -1e30)
                        vals = spw.tile([8, 32], bf16, tag="vals",
                                        name="vals")
                        idxu = spw.tile([8, 32], mybir.dt.uint16, tag="idxu",
                                        name="idxu")
                        for r in range(4):
                            nc.vector.max(vals[:, 8 * r:8 * r + 8], h4r[:])
                            nc.vector.max_index(idxu[:, 8 * r:8 * r + 8],
                                                vals[:, 8 * r:8 * r + 8],
                                                h4r[:])
                            nc.vector.match_replace(h4r[:],
                                                    vals[:, 8 * r:8 * r + 8],
                                                    h4r[:], -1e30)
                        goff = spw.tile([8, 32], mybir.dt.uint16, tag="goff",
                                        name="goff")
                        nc.gpsimd.iota(goff[:], pattern=[[0, 32]], base=0,
                                       channel_multiplier=200)
                        nc.vector.tensor_tensor(out=idxu[:], in0=idxu[:],
                                                in1=goff[:], op=OP.add)
                        nc.sync.dma_start(
                            idxbuf_d.ap()[256 * q:256 * q + 256]
                            .rearrange("(g k) -> g k", g=8),
                            idxu[:].bitcast(mybir.dt.int16))
                        idxw = spw.tile([112, 16], mybir.dt.int16, tag="idxw",
                                        name="idxw")
                        src = (idxbuf_d.ap()[256 * q:256 * q + 256]
                               .rearrange("(c p) -> p c", p=16))
                        for rep in range(7):
                            eng = (nc.sync, nc.scalar)[rep % 2]
                            eng.dma_start(idxw[16 * rep:16 * rep + 16, :],
                                          src)
                        nc.gpsimd.ap_gather(topsT[:, 256 * q:256 * q + 256],
                                            hcat32[:, s0:s0 + 1600],
                                            idxw[:], channels=112,
                                            num_elems=1600, d=1,
                                            num_idxs=256)
                        nc.gpsimd.tensor_copy(topsB[:, 256 * q:256 * q + 256],
                                               topsT[:, 256 * q:256 * q + 256])

                    # prologue: deg (32 rows) chunks -> rsqrt -> replicate
                    def deg_chunk(q):
                        c0, c1 = CW * q, CW * (q + 1)
                        dstage = stp.tile([32, CW], bf16, tag="dstage",
                                          name="dstage")
                        nc.scalar.dma_start(dstage[:], deg32_d.ap()[:, c0:c1])
                        nc.scalar.activation(dinv_rep[0:32, c0:c1],
                                             dstage[:],
                                             AF.Abs_reciprocal_sqrt)
                        for rep in range(1, 4):
                            nc.sync.dma_start(
                                dinv_rep[32 * rep:32 * rep + 32, c0:c1],
                                dinv_rep[0:32, c0:c1])

                    xstash = {}

                    def x_chunk(h):
                        xs = xsp.tile([128, 800], bf16, tag="xs", name="xs")
                        nc.gpsimd.dma_start(
                            xs[:], xT_d.ap()[:, 800 * h:800 * (h + 1)])
                        xstash[h] = xs

                    tiles = {0: stage_chunk(0), 1: stage_chunk(1)}
                    x_chunk(0)
                    x_chunk(1)
                    deg_chunk(0)
                    deg_chunk(1)
                    scale_chunk(0, tiles.pop(0))

                    for q in range(8):
                        if q + 2 < 8:
                            tiles[q + 2] = stage_chunk(q + 2)
                            deg_chunk(q + 2)
                        if q + 1 < 8:
                            scale_chunk(q + 1, tiles.pop(q + 1))
                        if 2 * q + 2 < 16:
                            x_chunk(2 * q + 2)
                        if 2 * q + 3 < 16:
                            x_chunk(2 * q + 3)
                        for l in range(4):
                            for r in range(4):
                                pair_layer(4 * q + r, l)
                                if l == 3:
                                    gp = 4 * q + r
                                    nc.vector.tensor_copy(
                                        hcat32[:, 400 * gp:400 * gp + 400],
                                        hcat[:, 400 * gp:400 * gp + 400])
                        sortpool_group(q)
                        if q == 1:
                            load_head_consts()
                        xstash.pop(2 * q, None)
                        xstash.pop(2 * q + 1, None)

            # ---- head: conv1(97->16) -> maxpool2 -> conv2(16->32,k=5)
            #      -> fc 352->128 -> fc 128->1 ----
            with (
                tc.tile_pool(name="head", bufs=2) as hp,
                tc.tile_pool(name="psH", bufs=1, space="PSUM") as psH,
            ):
                for q in range(4):
                    pc1 = psH.tile([16, 480], fp32, tag="c1", bufs=2,
                                   name="pc1")
                    rhs = _apf(topsB[0:97, 512 * q:512 * q + 1],
                               [[32, 16], [1, 30]])
                    nc.tensor.matmul(pc1[:], lhsT=cw1[:], rhs=rhs,
                                     start=True, stop=True)
                    nc.scalar.activation(c1T[:, 480 * q:480 * q + 480],
                                         pc1[:], AF.Relu, bias=cb1[:])
                poolT = hp.tile([16, 15 * GPC], bf16, tag="poolT")
                nc.vector.tensor_tensor(
                    out=_apf(poolT[0:16, 0:1], [[15, GPC], [1, 15]]),
                    in0=_apf(c1T[0:16, 0:1], [[30, GPC], [2, 15]]),
                    in1=_apf(c1T[0:16, 1:2], [[30, GPC], [2, 15]]),
                    op=OP.max)
                c2Te = hp.tile([33, 11 * GPC], bf16, tag="c2T")
                nc.vector.memset(c2Te[32:33, :], 1.0)
                for q in range(2):
                    pc2 = psH.tile([32, 352], fp32, tag="c2", bufs=2,
                                   name="pc2")
                    for t in range(5):
                        rhs = _apf(poolT[0:16, 480 * q + t:480 * q + t + 1],
                                   [[15, 32], [1, 11]])
                        nc.tensor.matmul(pc2[:], lhsT=cw2[t][:], rhs=rhs,
                                         start=(t == 0), stop=(t == 4))
                    nc.scalar.activation(c2Te[0:32, 352 * q:352 * q + 352],
                                         pc2[:], AF.Relu, bias=cb2[:])
                # fc1 graph-major: hlin[g, o2] via 11 accumulating matmuls
                # over conv positions; bias rides the ones row of c2Te.
                ph = psH.tile([64, 128], fp32, tag="hl")
                for p in range(11):
                    nc.tensor.matmul(ph[:],
                                     lhsT=_apf(c2Te[0:33, p:p + 1],
                                               [[11, GPC]]),
                                     rhs=lw1p[:, 128 * p:128 * p + 128],
                                     start=(p == 0), stop=(p == 10))
                hlin = hp.tile([64, 128], bf16, tag="hlin")
                nc.scalar.activation(hlin[:], ph[:], AF.Relu)
                pt = psH.tile([128, 64], bf16, tag="pT", name="pt")
                nc.tensor.transpose(pt[:], hlin[:], identb[:])
                hlinT = hp.tile([128, 64], bf16, tag="hlinT")
                nc.vector.tensor_copy(hlinT[:], pt[:])
                po = psH.tile([1, 64], fp32, tag="po")
                nc.tensor.matmul(po[:], lhsT=lw2[:], rhs=hlinT[:],
                                 start=True, stop=True)
                # sigmoid(z+b) = 0.5 + 0.5*tanh(0.5*(z+b)); reuses the tanh
                # table already loaded, skipping a 1.3us ACT_TABLE_LOAD
                lb2h = hp.tile([1, 1], fp32, tag="lb2h")
                nc.vector.tensor_scalar(out=lb2h[:], in0=lb2[:],
                                        scalar1=0.5, scalar2=None,
                                        op0=OP.mult)
                outT = hp.tile([1, 64], fp32, tag="outT")
                nc.scalar.activation(outT[:], po[:], AF.Tanh,
                                     bias=lb2h[:], scale=0.5)
                outS = hp.tile([1, 64], fp32, tag="outS")
                nc.vector.tensor_scalar(out=outS[:], in0=outT[:],
                                        scalar1=0.5, scalar2=0.5,
                                        op0=OP.mult, op1=OP.add)
                nc.sync.dma_start(out_d.ap(), outS[:])

    nc.compile()
    return nc


def _lw1p(lw1, lb1):
    """[352,128] fc1 weight -> [33, 11*128]: lw1p[o, 128p+o2] =
    lw1[11o+p, o2]; row 32 carries the bias (paired with the ones row of
    c2Te, emitted only in the p=0 slice so it is added exactly once)."""
    out = np.zeros((33, 11 * 128), np.float32)
    r = lw1.reshape(32, 11, 128)
    for p in range(11):
        out[0:32, 128 * p:128 * (p + 1)] = r[:, p, :]
    out[32, 0:128] = lb1
    return out


def _prep_inputs(inputs):
    """Shard + densify on host. Returns per-core in_maps."""
    import ml_dtypes
    bf = ml_dtypes.bfloat16
    x = np.asarray(inputs["x"], np.float32)
    ei = np.asarray(inputs["edge_index"], np.int64)
    src, dst = ei[0], ei[1]
    g_edge = dst // M
    jl = src - g_edge * M
    il = dst - g_edge * M
    flat = g_edge * (M * M) + jl * M + il
    cnt = np.bincount(flat, minlength=B * M * M).astype(np.float32)
    adj = cnt.reshape(B, M, M)
    adj += np.eye(M, dtype=np.float32)[None]
    deg = adj.sum(axis=1)                      # [B, M] integer-valued

    w234 = np.concatenate(
        [np.asarray(inputs["W2"], np.float32),
         np.asarray(inputs["W3"], np.float32),
         np.pad(np.asarray(inputs["W4"], np.float32), ((0, 0), (0, 31)))],
        axis=0)  # [96, 32]
    b4p = np.pad(np.asarray(inputs["b4"], np.float32), (0, 31))
    bgcn = np.stack(
        [np.asarray(inputs["b1"], np.float32),
         np.asarray(inputs["b2"], np.float32),
         np.asarray(inputs["b3"], np.float32), b4p], axis=1)  # [32, 4]
    cw1 = np.ascontiguousarray(
        np.asarray(inputs["convW1"], np.float32)[:, 0, :].T)  # [97,16]
    cw2_r = np.asarray(inputs["convW2"], np.float32)  # [32,16,5]
    cw2 = np.ascontiguousarray(
        cw2_r.transpose(2, 1, 0).reshape(80, 32))  # [(t,i),o]
    common = {
        "w1": np.asarray(inputs["W1"], np.float32).astype(bf),
        "w234": np.ascontiguousarray(w234).astype(bf),
        "bgcn": np.ascontiguousarray(bgcn),
        "cw1": cw1.astype(bf),
        "cb1": np.asarray(inputs["convb1"], np.float32).reshape(16, 1),
        "cw2": cw2.astype(bf),
        "cb2": np.asarray(inputs["convb2"], np.float32).reshape(32, 1),
        "lw1p": _lw1p(np.asarray(inputs["linW1"], np.float32),
                      np.asarray(inputs["linb1"], np.float32)).astype(bf),
        "lw2": np.asarray(inputs["linW2"], np.float32).astype(bf),
        "lb2": np.asarray(inputs["linb2"], np.float32).reshape(1, 1),
    }
    in_maps = []
    for c in range(NCORES):
        m = dict(common)
        xc = x[NPC * c:NPC * (c + 1)]
        m["xT"] = np.ascontiguousarray(xc.T).astype(bf)
        adjc = adj[GPC * c:GPC * (c + 1)]            # [64, 200, 200]
        m["adjlo"] = np.ascontiguousarray(
            adjc[:, 0:128, :].transpose(1, 0, 2).reshape(128, NPC)).astype(bf)
        m["adjhi"] = np.ascontiguousarray(
            adjc[:, 128:200, :].transpose(1, 0, 2).reshape(72, NPC)).astype(bf)
        degc = deg[GPC * c:GPC * (c + 1)]            # [64, 200]
        m["deg32"] = np.ascontiguousarray(
            np.broadcast_to(degc.reshape(1, NPC), (32, NPC))).astype(bf)
        dcol = np.ones((128, 128), np.float32)
        dcol[:, 0::2] = degc[:, 0:128].T             # col 2g <- nodes 0:128
        dcol[0:72, 1::2] = degc[:, 128:200].T        # col 2g+1 <- nodes hi
        m["degcol"] = dcol.astype(bf)
        in_maps.append(m)
    return in_maps


def _run(inputs, trace=False):
    from concourse import bass_utils
    if "nc" not in _STATE:
        _STATE["nc"] = _build()
    nc = _STATE["nc"]
    in_maps = _prep_inputs(inputs)
    res = bass_utils.run_bass_kernel_spmd(
        nc, in_maps, core_ids=list(range(NCORES)), trace=trace)
    out = np.concatenate([res.results[c]["out"].reshape(GPC)
                          for c in range(NCORES)])
    return out.reshape(B, 1).astype(np.float32), res


def kernel(**inputs) -> np.ndarray:
    out, _ = _run(inputs, trace=False)
    return out



# revision 43
# speedup vs baseline: 1.0833x; 1.0833x over previous
"""DGCNN (4x GCNConv + SortPool + Conv1d head) on 8 Trainium2 NeuronCores.

Data-parallel over graphs: each core owns 64 of the 512 graphs.
Per graph the GCN aggregation is computed densely in bf16:
    agg^T[f, i] = sum_j (y[j, f] * dinv[j]) * (adj[j, i] * dinv[i])
with adj the src-major dense adjacency-with-self-loops count matrix,
densified on the host from edge_index (a re-layout of the integer graph
structure); integer degree counts are likewise host re-layouts. All
floating-point math (rsqrt normalization, 4 GCN layers, SortPool top-k
selection+gather, conv/MLP head) runs on-device. Matmul operands are
bf16 (exact for the integer-valued adjacency), accumulation is fp32.
"""

import numpy as np

B = 512
M = 200
GPC = 64            # graphs per core
NPC = GPC * M       # nodes per core
NCORES = 8
K = 30
F = 97

_STATE = {}


def _apf(base, pairs):
    """AP with the partition dim of `base` and custom free [step,count] pairs."""
    import concourse.bass as bass
    return bass.AP(tensor=base.tensor, offset=base.offset,
                   ap=[list(base.ap[0])] + [list(p) for p in pairs])


def _build(skip=()):
    skip = set(skip)
    import concourse.bass as bass
    import concourse.bacc as bacc
    import concourse.mybir as mybir
    from concourse.tile import TileContext
    from concourse.masks import make_identity

    fp32 = mybir.dt.float32
    bf16 = mybir.dt.bfloat16
    AF = mybir.ActivationFunctionType
    OP = mybir.AluOpType

    nc = bacc.Bacc("TRN2", target_bir_lowering=False, debug=False,
                   num_devices=NCORES)

    xT_d = nc.dram_tensor("xT", [128, NPC], bf16, kind="ExternalInput")
    adjlo_d = nc.dram_tensor("adjlo", [128, NPC], bf16, kind="ExternalInput")
    adjhi_d = nc.dram_tensor("adjhi", [72, NPC], bf16, kind="ExternalInput")
    deg32_d = nc.dram_tensor("deg32", [32, NPC], bf16, kind="ExternalInput")
    degcol_d = nc.dram_tensor("degcol", [128, 128], bf16, kind="ExternalInput")
    w1_d = nc.dram_tensor("w1", [128, 32], bf16, kind="ExternalInput")
    w234_d = nc.dram_tensor("w234", [96, 32], bf16, kind="ExternalInput")
    bgcn_d = nc.dram_tensor("bgcn", [32, 4], fp32, kind="ExternalInput")
    cw1_d = nc.dram_tensor("cw1", [97, 16], bf16, kind="ExternalInput")
    cb1_d = nc.dram_tensor("cb1", [16, 1], fp32, kind="ExternalInput")
    cw2_d = nc.dram_tensor("cw2", [80, 32], bf16, kind="ExternalInput")
    cb2_d = nc.dram_tensor("cb2", [32, 1], fp32, kind="ExternalInput")
    lw1p_d = nc.dram_tensor("lw1p", [33, 1408], bf16, kind="ExternalInput")
    lw2_d = nc.dram_tensor("lw2", [128, 1], bf16, kind="ExternalInput")
    lb2_d = nc.dram_tensor("lb2", [1, 1], fp32, kind="ExternalInput")

    idxbuf_d = nc.dram_tensor("idxbuf", [GPC * 32], mybir.dt.int16,
                              kind="Internal")
    out_d = nc.dram_tensor("out", [1, GPC], fp32, kind="ExternalOutput")

    NCH = 8                 # prologue chunks
    CW = NPC // NCH         # 1600 cols per chunk

    with TileContext(nc) as tc:
        with tc.tile_pool(name="const", bufs=1) as cp:
            identb = cp.tile([64, 64], bf16)
            make_identity(nc, identb[:])
            w1 = cp.tile([128, 32], bf16)
            nc.sync.dma_start(w1[:], w1_d.ap())
            w234 = cp.tile([96, 32], bf16)
            nc.sync.dma_start(w234[:], w234_d.ap())
            bgcn = cp.tile([32, 4], fp32)
            nc.sync.dma_start(bgcn[:], bgcn_d.ap())
            cw1 = cp.tile([97, 16], bf16)
            cb1 = cp.tile([16, 1], fp32)
            cw2 = [cp.tile([16, 32], bf16, tag=f"cw2_{t}", name=f"cw2_{t}")
                   for t in range(5)]
            cb2 = cp.tile([32, 1], fp32)
            lw1p = cp.tile([33, 1408], bf16)
            lw2 = cp.tile([128, 1], bf16)
            lb2 = cp.tile([1, 1], fp32)

            def load_head_consts():
                # deferred: head weights aren't needed until the tail, so
                # keep their DMAs out of the prologue's sync queue
                nc.sync.dma_start(cw1[:], cw1_d.ap())
                nc.sync.dma_start(cb1[:], cb1_d.ap())
                for t in range(5):
                    nc.sync.dma_start(cw2[t][:],
                                      cw2_d.ap()[16 * t:16 * t + 16, :])
                nc.sync.dma_start(cb2[:], cb2_d.ap())
                nc.sync.dma_start(lw1p[:], lw1p_d.ap())
                nc.sync.dma_start(lw2[:], lw2_d.ap())
                nc.sync.dma_start(lb2[:], lb2_d.ap())

            # per-node 1/sqrt(deg) columns: col 2g = nodes 0:128 of graph g,
            # col 2g+1 = nodes 128:200 (on partitions 0:72)
            dinv_col = cp.tile([128, 128], fp32)
            dcol_s = cp.tile([128, 128], bf16, name="dcol_s")
            nc.sync.dma_start(dcol_s[:], degcol_d.ap())
            nc.scalar.activation(dinv_col[:], dcol_s[:],
                                 AF.Abs_reciprocal_sqrt)

            # hcat rows: 0:32 h1, 32:64 h2, 64:96 h3, 96 h4; col = 256*g + i
            hcat = cp.tile([112, 200 * GPC], bf16)
            topsT = cp.tile([112, 32 * GPC], fp32)
            topsB = cp.tile([112, 32 * GPC], bf16)
            c1T = cp.tile([16, 30 * GPC], bf16)

            if "agg" in skip:
                nc.gpsimd.memset(hcat[:], 0.25)

            with tc.tile_pool(name="gcn", bufs=1) as gp_pool:
                adjS_lo = gp_pool.tile([128, NPC], bf16)
                adjS_hi = gp_pool.tile([72, NPC], bf16)
                dinv_rep = gp_pool.tile([128, NPC], bf16)
                hcat32 = gp_pool.tile([112, 200 * GPC], fp32)

                with (
                    tc.tile_pool(name="stage", bufs=3) as stp,
                    tc.tile_pool(name="work", bufs=4) as wp,
                    tc.tile_pool(name="spw", bufs=2) as spw,
                    tc.tile_pool(name="xsp", bufs=4) as xsp,
                    tc.tile_pool(name="psY", bufs=4, space="PSUM") as psY,
                    tc.tile_pool(name="psG", bufs=4, space="PSUM") as psG,
                ):
                    def stage_chunk(q):
                        c0, c1 = CW * q, CW * (q + 1)
                        astage = stp.tile([128, CW], bf16, tag="astage",
                                          name="astage")
                        nc.sync.dma_start(astage[:], adjlo_d.ap()[:, c0:c1])
                        hstage = stp.tile([72, CW], bf16, tag="hstage",
                                          name="hstage")
                        nc.scalar.dma_start(hstage[:], adjhi_d.ap()[:, c0:c1])
                        return astage, hstage

                    def scale_chunk(q, tiles):
                        c0, c1 = CW * q, CW * (q + 1)
                        astage, hstage = tiles
                        nc.vector.tensor_tensor(
                            out=adjS_lo[:, c0:c1], in0=astage[:],
                            in1=dinv_rep[:, c0:c1], op=OP.mult)
                        nc.vector.tensor_tensor(
                            out=adjS_hi[:, c0:c1], in0=hstage[:],
                            in1=dinv_rep[0:72, c0:c1], op=OP.mult)

                    def pair_layer(gp, l):
                        fo = 32 if l < 3 else 1
                        py = psY.tile([128, 128], fp32, tag="y", name="py")
                        y = wp.tile([128, 128], bf16, tag="y_s", name="y")
                        for half in range(2):
                            g = 2 * gp + half
                            yo = 64 * half
                            if l == 0:
                                xs = xstash[gp // 2]
                                xo = 400 * (gp % 2) + 200 * half
                                lhs_lo = xs[:, xo:xo + 128]
                                lhs_hi = xs[:, xo + 128:xo + 200]
                                w_t = w1[:, 0:fo]
                            else:
                                r0 = 32 * (l - 1)
                                c0 = 200 * g
                                lhs_lo = hcat[r0:r0 + 32, c0:c0 + 128]
                                lhs_hi = hcat[r0:r0 + 32, c0 + 128:c0 + 200]
                                w_t = w234[r0:r0 + 32, 0:fo]
                            nc.tensor.matmul(py[:, yo:yo + fo],
                                             lhsT=lhs_lo, rhs=w_t,
                                             start=True, stop=True)
                            nc.tensor.matmul(py[0:72, yo + 32:yo + 32 + fo],
                                             lhsT=lhs_hi, rhs=w_t,
                                             start=True, stop=True)
                        # drain xw, row scale dinv[j]; blocks {lo, hi} x 2
                        nc.vector.tensor_tensor(
                            out=_apf(y[0:128, 0:1], [[64, 2], [1, fo]]),
                            in0=_apf(py[0:128, 0:1], [[64, 2], [1, fo]]),
                            in1=_apf(dinv_col[0:128, 4 * gp:4 * gp + 1],
                                     [[2, 2], [0, fo]]),
                            op=OP.mult)
                        nc.vector.tensor_tensor(
                            out=_apf(y[0:72, 32:33], [[64, 2], [1, fo]]),
                            in0=_apf(py[0:72, 32:33], [[64, 2], [1, fo]]),
                            in1=_apf(dinv_col[0:72,
                                     4 * gp + 1:4 * gp + 2],
                                     [[2, 2], [0, fo]]),
                            op=OP.mult)
                        pagg = psG.tile([32, 456], fp32, tag="agg",
                                        name="pagg")
                        for half in range(2):
                            g = 2 * gp + half
                            yo, co = 64 * half, 256 * half
                            nc.tensor.matmul(
                                pagg[0:fo, co:co + 200],
                                lhsT=y[:, yo:yo + fo],
                                rhs=adjS_lo[:, 200 * g:200 * g + 200],
                                start=True, stop=False)
                            nc.tensor.matmul(
                                pagg[0:fo, co:co + 200],
                                lhsT=y[0:72, yo + 32:yo + 32 + fo],
                                rhs=adjS_hi[:, 200 * g:200 * g + 200],
                                start=False, stop=True)
                        r0 = 32 * l if l < 3 else 96
                        nc.scalar.activation(
                            hcat[r0:r0 + fo, 400 * gp:400 * gp + 400],
                            _apf(pagg[0:fo, 0:1], [[256, 2], [1, 200]]),
                            AF.Tanh, bias=bgcn[0:fo, l:l + 1])

                    def sortpool_group(q):
                        # top-30 for graphs 8q..8q+8; cols 1600q..1600q+1600
                        s0 = 1600 * q
                        h4r = spw.tile([8, 256], bf16, tag="h4r", name="h4r")
                        nc.sync.dma_start(
                            h4r[:, 0:200],
                            _apf(hcat[96:97, s0:s0 + 1],
                                 [[200, 8], [1, 200]]))
                        nc.vector.memset(h4r[:, 200:256], -1e30)
                        vals = spw.tile([8, 32], bf16, tag="vals",
                                        name="vals")
                        idxu = spw.tile([8, 32], mybir.dt.uint16, tag="idxu",
                                        name="idxu")
                        for r in range(4):
                            nc.vector.max(vals[:, 8 * r:8 * r + 8], h4r[:])
                            nc.vector.max_index(idxu[:, 8 * r:8 * r + 8],
                                                vals[:, 8 * r:8 * r + 8],
                                                h4r[:])
                            nc.vector.match_replace(h4r[:],
                                                    vals[:, 8 * r:8 * r + 8],
                                                    h4r[:], -1e30)
                        goff = spw.tile([8, 32], mybir.dt.uint16, tag="goff",
                                        name="goff")
                        nc.gpsimd.iota(goff[:], pattern=[[0, 32]], base=0,
                                       channel_multiplier=200)
                        nc.vector.tensor_tensor(out=idxu[:], in0=idxu[:],
                                                in1=goff[:], op=OP.add)
                        nc.sync.dma_start(
                            idxbuf_d.ap()[256 * q:256 * q + 256]
                            .rearrange("(g k) -> g k", g=8),
                            idxu[:].bitcast(mybir.dt.int16))
                        idxw = spw.tile([112, 16], mybir.dt.int16, tag="idxw",
                                        name="idxw")
                        src = (idxbuf_d.ap()[256 * q:256 * q + 256]
                               .rearrange("(c p) -> p c", p=16))
                        for rep in range(7):
                            nc.sync.dma_start(idxw[16 * rep:16 * rep + 16, :],
                                              src)
                        nc.gpsimd.ap_gather(topsT[:, 256 * q:256 * q + 256],
                                            hcat32[:, s0:s0 + 1600],
                                            idxw[:], channels=112,
                                            num_elems=1600, d=1,
                                            num_idxs=256)
                        nc.gpsimd.tensor_copy(topsB[:, 256 * q:256 * q + 256],
                                               topsT[:, 256 * q:256 * q + 256])

                    # prologue: deg (32 rows) chunks -> rsqrt -> replicate
                    def deg_chunk(q):
                        c0, c1 = CW * q, CW * (q + 1)
                        dstage = stp.tile([32, CW], bf16, tag="dstage",
                                          name="dstage")
                        nc.scalar.dma_start(dstage[:], deg32_d.ap()[:, c0:c1])
                        nc.scalar.activation(dinv_rep[0:32, c0:c1],
                                             dstage[:],
                                             AF.Abs_reciprocal_sqrt)
                        for rep in range(1, 4):
                            nc.sync.dma_start(
                                dinv_rep[32 * rep:32 * rep + 32, c0:c1],
                                dinv_rep[0:32, c0:c1])

                    xstash = {}

                    def x_chunk(h):
                        xs = xsp.tile([128, 800], bf16, tag="xs", name="xs")
                        nc.gpsimd.dma_start(
                            xs[:], xT_d.ap()[:, 800 * h:800 * (h + 1)])
                        xstash[h] = xs

                    tiles = {0: stage_chunk(0), 1: stage_chunk(1)}
                    x_chunk(0)
                    x_chunk(1)
                    deg_chunk(0)
                    deg_chunk(1)
                    scale_chunk(0, tiles.pop(0))

                    for q in range(8):
                        if q + 2 < 8:
                            tiles[q + 2] = stage_chunk(q + 2)
                            deg_chunk(q + 2)
                        if q + 1 < 8:
                            scale_chunk(q + 1, tiles.pop(q + 1))
                        if 2 * q + 2 < 16:
                            x_chunk(2 * q + 2)
                        if 2 * q + 3 < 16:
                            x_chunk(2 * q + 3)
                        for l in range(4):
                            for r in range(4):
                                pair_layer(4 * q + r, l)
                                if l == 3:
                                    gp = 4 * q + r
                                    nc.vector.tensor_copy(
                                        hcat32[:, 400 * gp:400 * gp + 400],
                                        hcat[:, 400 * gp:400 * gp + 400])
                        sortpool_group(q)
                        if q == 1:
                            load_head_consts()
                        xstash.pop(2 * q, None)
                        xstash.pop(2 * q + 1, None)

            # ---- head: conv1(97->16) -> maxpool2 -> conv2(16->32,k=5)
            #      -> fc 352->128 -> fc 128->1 ----
            with (
                tc.tile_pool(name="head", bufs=2) as hp,
                tc.tile_pool(name="psH", bufs=1, space="PSUM") as psH,
            ):
                for q in range(4):
                    pc1 = psH.tile([16, 480], fp32, tag="c1", bufs=2,
                                   name="pc1")
                    rhs = _apf(topsB[0:97, 512 * q:512 * q + 1],
                               [[32, 16], [1, 30]])
                    nc.tensor.matmul(pc1[:], lhsT=cw1[:], rhs=rhs,
                                     start=True, stop=True)
                    nc.scalar.activation(c1T[:, 480 * q:480 * q + 480],
                                         pc1[:], AF.Relu, bias=cb1[:])
                poolT = hp.tile([16, 15 * GPC], bf16, tag="poolT")
                nc.vector.tensor_tensor(
                    out=_apf(poolT[0:16, 0:1], [[15, GPC], [1, 15]]),
                    in0=_apf(c1T[0:16, 0:1], [[30, GPC], [2, 15]]),
                    in1=_apf(c1T[0:16, 1:2], [[30, GPC], [2, 15]]),
                    op=OP.max)
                c2Te = hp.tile([33, 11 * GPC], bf16, tag="c2T")
                nc.vector.memset(c2Te[32:33, :], 1.0)
                for q in range(2):
                    pc2 = psH.tile([32, 352], fp32, tag="c2", bufs=2,
                                   name="pc2")
                    for t in range(5):
                        rhs = _apf(poolT[0:16, 480 * q + t:480 * q + t + 1],
                                   [[15, 32], [1, 11]])
                        nc.tensor.matmul(pc2[:], lhsT=cw2[t][:], rhs=rhs,
                                         start=(t == 0), stop=(t == 4))
                    nc.scalar.activation(c2Te[0:32, 352 * q:352 * q + 352],
                                         pc2[:], AF.Relu, bias=cb2[:])
                # fc1 graph-major: hlin[g, o2] via 11 accumulating matmuls
                # over conv positions; bias rides the ones row of c2Te.
                ph = psH.tile([64, 128], fp32, tag="hl")
                for p in range(11):
                    nc.tensor.matmul(ph[:],
                                     lhsT=_apf(c2Te[0:33, p:p + 1],
                                               [[11, GPC]]),
                                     rhs=lw1p[:, 128 * p:128 * p + 128],
                                     start=(p == 0), stop=(p == 10))
                hlin = hp.tile([64, 128], bf16, tag="hlin")
                nc.scalar.activation(hlin[:], ph[:], AF.Relu)
                pt = psH.tile([128, 64], bf16, tag="pT", name="pt")
                nc.tensor.transpose(pt[:], hlin[:], identb[:])
                hlinT = hp.tile([128, 64], bf16, tag="hlinT")
                nc.vector.tensor_copy(hlinT[:], pt[:])
                po = psH.tile([1, 64], fp32, tag="po")
                nc.tensor.matmul(po[:], lhsT=lw2[:], rhs=hlinT[:],
                                 start=True, stop=True)
                # sigmoid(z+b) = 0.5 + 0.5*tanh(0.5*(z+b)); reuses the tanh
                # table already loaded, skipping a 1.3us ACT_TABLE_LOAD
                lb2h = hp.tile([1, 1], fp32, tag="lb2h")
                nc.vector.tensor_scalar(out=lb2h[:], in0=lb2[:],
                                        scalar1=0.5, scalar2=None,
                                        op0=OP.mult)
                outT = hp.tile([1, 64], fp32, tag="outT")
                nc.scalar.activation(outT[:], po[:], AF.Tanh,
                                     bias=lb2h[:], scale=0.5)
                outS = hp.tile([1, 64], fp32, tag="outS")
                nc.vector.tensor_scalar(out=outS[:], in0=outT[:],
                                        scalar1=0.5, scalar2=0.5,
                                        op0=OP.mult, op1=OP.add)
                nc.sync.dma_start(out_d.ap(), outS[:])

    nc.compile()
    return nc


def _lw1p(lw1, lb1):
    """[352,128] fc1 weight -> [33, 11*128]: lw1p[o, 128p+o2] =
    lw1[11o+p, o2]; row 32 carries the bias (paired with the ones row of
    c2Te, emitted only in the p=0 slice so it is added exactly once)."""
    out = np.zeros((33, 11 * 128), np.float32)
    r = lw1.reshape(32, 11, 128)
    for p in range(11):
        out[0:32, 128 * p:128 * (p + 1)] = r[:, p, :]
    out[32, 0:128] = lb1
    return out


def _prep_inputs(inputs):
    """Shard + densify on host. Returns per-core in_maps."""
    import ml_dtypes
    bf = ml_dtypes.bfloat16
    x = np.asarray(inputs["x"], np.float32)
    ei = np.asarray(inputs["edge_index"], np.int64)
    src, dst = ei[0], ei[1]
    g_edge = dst // M
    jl = src - g_edge * M
    il = dst - g_edge * M
    flat = g_edge * (M * M) + jl * M + il
    cnt = np.bincount(flat, minlength=B * M * M).astype(np.float32)
    adj = cnt.reshape(B, M, M)
    adj += np.eye(M, dtype=np.float32)[None]
    deg = adj.sum(axis=1)                      # [B, M] integer-valued

    w234 = np.concatenate(
        [np.asarray(inputs["W2"], np.float32),
         np.asarray(inputs["W3"], np.float32),
         np.pad(np.asarray(inputs["W4"], np.float32), ((0, 0), (0, 31)))],
        axis=0)  # [96, 32]
    b4p = np.pad(np.asarray(inputs["b4"], np.float32), (0, 31))
    bgcn = np.stack(
        [np.asarray(inputs["b1"], np.float32),
         np.asarray(inputs["b2"], np.float32),
         np.asarray(inputs["b3"], np.float32), b4p], axis=1)  # [32, 4]
    cw1 = np.ascontiguousarray(
        np.asarray(inputs["convW1"], np.float32)[:, 0, :].T)  # [97,16]
    cw2_r = np.asarray(inputs["convW2"], np.float32)  # [32,16,5]
    cw2 = np.ascontiguousarray(
        cw2_r.transpose(2, 1, 0).reshape(80, 32))  # [(t,i),o]
    common = {
        "w1": np.asarray(inputs["W1"], np.float32).astype(bf),
        "w234": np.ascontiguousarray(w234).astype(bf),
        "bgcn": np.ascontiguousarray(bgcn),
        "cw1": cw1.astype(bf),
        "cb1": np.asarray(inputs["convb1"], np.float32).reshape(16, 1),
        "cw2": cw2.astype(bf),
        "cb2": np.asarray(inputs["convb2"], np.float32).reshape(32, 1),
        "lw1p": _lw1p(np.asarray(inputs["linW1"], np.float32),
                      np.asarray(inputs["linb1"], np.float32)).astype(bf),
        "lw2": np.asarray(inputs["linW2"], np.float32).astype(bf),
        "lb2": np.asarray(inputs["linb2"], np.float32).reshape(1, 1),
    }
    in_maps = []
    for c in range(NCORES):
        m = dict(common)
        xc = x[NPC * c:NPC * (c + 1)]
        m["xT"] = np.ascontiguousarray(xc.T).astype(bf)
        adjc = adj[GPC * c:GPC * (c + 1)]            # [64, 200, 200]
        m["adjlo"] = np.ascontiguousarray(
            adjc[:, 0:128, :].transpose(1, 0, 2).reshape(128, NPC)).astype(bf)
        m["adjhi"] = np.ascontiguousarray(
            adjc[:, 128:200, :].transpose(1, 0, 2).reshape(72, NPC)).astype(bf)
        degc = deg[GPC * c:GPC * (c + 1)]            # [64, 200]
        m["deg32"] = np.ascontiguousarray(
            np.broadcast_to(degc.reshape(1, NPC), (32, NPC))).astype(bf)
        dcol = np.ones((128, 128), np.float32)
        dcol[:, 0::2] = degc[:, 0:128].T             # col 2g <- nodes 0:128
        dcol[0:72, 1::2] = degc[:, 128:200].T        # col 2g+1 <- nodes hi
        m["degcol"] = dcol.astype(bf)
        in_maps.append(m)
    return in_maps


def _run(inputs, trace=False):
    from concourse import bass_utils
    if "nc" not in _STATE:
        _STATE["nc"] = _build()
    nc = _STATE["nc"]
    in_maps = _prep_inputs(inputs)
    res = bass_utils.run_bass_kernel_spmd(
        nc, in_maps, core_ids=list(range(NCORES)), trace=trace)
    out = np.concatenate([res.results[c]["out"].reshape(GPC)
                          for c in range(NCORES)])
    return out.reshape(B, 1).astype(np.float32), res


def kernel(**inputs) -> np.ndarray:
    out, _ = _run(inputs, trace=False)
    return out



# revision 44
# speedup vs baseline: 1.1124x; 1.0268x over previous
"""DGCNN (4x GCNConv + SortPool + Conv1d head) on 8 Trainium2 NeuronCores.

Data-parallel over graphs: each core owns 64 of the 512 graphs.
Per graph the GCN aggregation is computed densely in bf16:
    agg^T[f, i] = sum_j (y[j, f] * dinv[j]) * (adj[j, i] * dinv[i])
with adj the src-major dense adjacency-with-self-loops count matrix,
densified on the host from edge_index (a re-layout of the integer graph
structure); integer degree counts are likewise host re-layouts. All
floating-point math (rsqrt normalization, 4 GCN layers, SortPool top-k
selection+gather, conv/MLP head) runs on-device. Matmul operands are
bf16 (exact for the integer-valued adjacency), accumulation is fp32.
"""

import numpy as np

B = 512
M = 200
GPC = 64            # graphs per core
NPC = GPC * M       # nodes per core
NCORES = 8
K = 30
F = 97

_STATE = {}


def _apf(base, pairs):
    """AP with the partition dim of `base` and custom free [step,count] pairs."""
    import concourse.bass as bass
    return bass.AP(tensor=base.tensor, offset=base.offset,
                   ap=[list(base.ap[0])] + [list(p) for p in pairs])


def _build(skip=()):
    skip = set(skip)
    import concourse.bass as bass
    import concourse.bacc as bacc
    import concourse.mybir as mybir
    from concourse.tile import TileContext
    from concourse.masks import make_identity

    fp32 = mybir.dt.float32
    bf16 = mybir.dt.bfloat16
    AF = mybir.ActivationFunctionType
    OP = mybir.AluOpType

    nc = bacc.Bacc("TRN2", target_bir_lowering=False, debug=False,
                   num_devices=NCORES)

    xT_d = nc.dram_tensor("xT", [128, NPC], bf16, kind="ExternalInput")
    adjlo_d = nc.dram_tensor("adjlo", [128, NPC], bf16, kind="ExternalInput")
    adjhi_d = nc.dram_tensor("adjhi", [72, NPC], bf16, kind="ExternalInput")
    deg32_d = nc.dram_tensor("deg32", [32, NPC], bf16, kind="ExternalInput")
    degcol_d = nc.dram_tensor("degcol", [128, 128], bf16, kind="ExternalInput")
    w1_d = nc.dram_tensor("w1", [128, 32], bf16, kind="ExternalInput")
    w234_d = nc.dram_tensor("w234", [96, 32], bf16, kind="ExternalInput")
    bgcn_d = nc.dram_tensor("bgcn", [32, 4], fp32, kind="ExternalInput")
    cw1_d = nc.dram_tensor("cw1", [97, 16], bf16, kind="ExternalInput")
    cb1_d = nc.dram_tensor("cb1", [16, 1], fp32, kind="ExternalInput")
    cw2_d = nc.dram_tensor("cw2", [80, 32], bf16, kind="ExternalInput")
    cb2_d = nc.dram_tensor("cb2", [32, 1], fp32, kind="ExternalInput")
    lw1p_d = nc.dram_tensor("lw1p", [33, 1408], bf16, kind="ExternalInput")
    lw2_d = nc.dram_tensor("lw2", [128, 1], bf16, kind="ExternalInput")
    lb2_d = nc.dram_tensor("lb2", [1, 1], fp32, kind="ExternalInput")

    idxbuf_d = nc.dram_tensor("idxbuf", [GPC * 32], mybir.dt.int16,
                              kind="Internal")
    out_d = nc.dram_tensor("out", [1, GPC], fp32, kind="ExternalOutput")

    NCH = 8                 # prologue chunks
    CW = NPC // NCH         # 1600 cols per chunk

    with TileContext(nc) as tc:
        with tc.tile_pool(name="const", bufs=1) as cp:
            identb = cp.tile([64, 64], bf16)
            make_identity(nc, identb[:])
            w1 = cp.tile([128, 32], bf16)
            nc.sync.dma_start(w1[:], w1_d.ap())
            w234 = cp.tile([96, 32], bf16)
            nc.sync.dma_start(w234[:], w234_d.ap())
            bgcn = cp.tile([32, 4], fp32)
            nc.sync.dma_start(bgcn[:], bgcn_d.ap())
            cw1 = cp.tile([97, 16], bf16)
            cb1 = cp.tile([16, 1], fp32)
            cw2 = [cp.tile([16, 32], bf16, tag=f"cw2_{t}", name=f"cw2_{t}")
                   for t in range(5)]
            cb2 = cp.tile([32, 1], fp32)
            lw1p = cp.tile([33, 1408], bf16)
            lw2 = cp.tile([128, 1], bf16)
            lb2 = cp.tile([1, 1], fp32)

            def load_head_consts():
                # deferred: head weights aren't needed until the tail, so
                # keep their DMAs out of the prologue's sync queue
                nc.sync.dma_start(cw1[:], cw1_d.ap())
                nc.sync.dma_start(cb1[:], cb1_d.ap())
                for t in range(5):
                    nc.sync.dma_start(cw2[t][:],
                                      cw2_d.ap()[16 * t:16 * t + 16, :])
                nc.sync.dma_start(cb2[:], cb2_d.ap())
                nc.sync.dma_start(lw1p[:], lw1p_d.ap())
                nc.sync.dma_start(lw2[:], lw2_d.ap())
                nc.sync.dma_start(lb2[:], lb2_d.ap())

            # per-node 1/sqrt(deg) columns: col 2g = nodes 0:128 of graph g,
            # col 2g+1 = nodes 128:200 (on partitions 0:72)
            dinv_col = cp.tile([128, 128], fp32)
            dcol_s = cp.tile([128, 128], bf16, name="dcol_s")
            nc.sync.dma_start(dcol_s[:], degcol_d.ap())
            nc.scalar.activation(dinv_col[:], dcol_s[:],
                                 AF.Abs_reciprocal_sqrt)

            # hcat rows: 0:32 h1, 32:64 h2, 64:96 h3, 96 h4; col = 256*g + i
            hcat = cp.tile([112, 200 * GPC], bf16)
            topsT = cp.tile([112, 32 * GPC], fp32)
            topsB = cp.tile([112, 32 * GPC], bf16)
            c1T = cp.tile([16, 30 * GPC], bf16)

            if "agg" in skip:
                nc.gpsimd.memset(hcat[:], 0.25)

            with tc.tile_pool(name="gcn", bufs=1) as gp_pool:
                adjS_lo = gp_pool.tile([128, NPC], bf16)
                adjS_hi = gp_pool.tile([72, NPC], bf16)
                dinv_rep = gp_pool.tile([128, NPC], bf16)
                hcat32 = gp_pool.tile([112, 200 * GPC], fp32)

                with (
                    tc.tile_pool(name="stage", bufs=3) as stp,
                    tc.tile_pool(name="work", bufs=4) as wp,
                    tc.tile_pool(name="spw", bufs=2) as spw,
                    tc.tile_pool(name="xsp", bufs=4) as xsp,
                    tc.tile_pool(name="psY", bufs=4, space="PSUM") as psY,
                    tc.tile_pool(name="psG", bufs=4, space="PSUM") as psG,
                ):
                    def stage_chunk(q):
                        c0, c1 = CW * q, CW * (q + 1)
                        astage = stp.tile([128, CW], bf16, tag="astage",
                                          name="astage")
                        nc.sync.dma_start(astage[:], adjlo_d.ap()[:, c0:c1])
                        hstage = stp.tile([72, CW], bf16, tag="hstage",
                                          name="hstage")
                        nc.scalar.dma_start(hstage[:], adjhi_d.ap()[:, c0:c1])
                        return astage, hstage

                    def scale_chunk(q, tiles):
                        c0, c1 = CW * q, CW * (q + 1)
                        astage, hstage = tiles
                        nc.vector.tensor_tensor(
                            out=adjS_lo[:, c0:c1], in0=astage[:],
                            in1=dinv_rep[:, c0:c1], op=OP.mult)
                        nc.vector.tensor_tensor(
                            out=adjS_hi[:, c0:c1], in0=hstage[:],
                            in1=dinv_rep[0:72, c0:c1], op=OP.mult)

                    def pair_layer(gp, l):
                        fo = 32 if l < 3 else 1
                        py = psY.tile([128, 128], fp32, tag="y", name="py")
                        y = wp.tile([128, 128], bf16, tag="y_s", name="y")
                        for half in range(2):
                            g = 2 * gp + half
                            yo = 64 * half
                            if l == 0:
                                xs = xstash[gp // 2]
                                xo = 400 * (gp % 2) + 200 * half
                                lhs_lo = xs[:, xo:xo + 128]
                                lhs_hi = xs[:, xo + 128:xo + 200]
                                w_t = w1[:, 0:fo]
                            else:
                                r0 = 32 * (l - 1)
                                c0 = 200 * g
                                lhs_lo = hcat[r0:r0 + 32, c0:c0 + 128]
                                lhs_hi = hcat[r0:r0 + 32, c0 + 128:c0 + 200]
                                w_t = w234[r0:r0 + 32, 0:fo]
                            nc.tensor.matmul(py[:, yo:yo + fo],
                                             lhsT=lhs_lo, rhs=w_t,
                                             start=True, stop=True)
                            nc.tensor.matmul(py[0:72, yo + 32:yo + 32 + fo],
                                             lhsT=lhs_hi, rhs=w_t,
                                             start=True, stop=True)
                        # drain xw, row scale dinv[j]; blocks {lo, hi} x 2
                        nc.vector.tensor_tensor(
                            out=_apf(y[0:128, 0:1], [[64, 2], [1, fo]]),
                            in0=_apf(py[0:128, 0:1], [[64, 2], [1, fo]]),
                            in1=_apf(dinv_col[0:128, 4 * gp:4 * gp + 1],
                                     [[2, 2], [0, fo]]),
                            op=OP.mult)
                        nc.vector.tensor_tensor(
                            out=_apf(y[0:72, 32:33], [[64, 2], [1, fo]]),
                            in0=_apf(py[0:72, 32:33], [[64, 2], [1, fo]]),
                            in1=_apf(dinv_col[0:72,
                                     4 * gp + 1:4 * gp + 2],
                                     [[2, 2], [0, fo]]),
                            op=OP.mult)
                        pagg = psG.tile([32, 456], fp32, tag="agg",
                                        name="pagg")
                        for half in range(2):
                            g = 2 * gp + half
                            yo, co = 64 * half, 256 * half
                            nc.tensor.matmul(
                                pagg[0:fo, co:co + 200],
                                lhsT=y[:, yo:yo + fo],
                                rhs=adjS_lo[:, 200 * g:200 * g + 200],
                                start=True, stop=False)
                            nc.tensor.matmul(
                                pagg[0:fo, co:co + 200],
                                lhsT=y[0:72, yo + 32:yo + 32 + fo],
                                rhs=adjS_hi[:, 200 * g:200 * g + 200],
                                start=False, stop=True)
                        r0 = 32 * l if l < 3 else 96
                        nc.scalar.activation(
                            hcat[r0:r0 + fo, 400 * gp:400 * gp + 400],
                            _apf(pagg[0:fo, 0:1], [[256, 2], [1, 200]]),
                            AF.Tanh, bias=bgcn[0:fo, l:l + 1])

                    def sortpool_group(q):
                        # top-30 for graphs 8q..8q+8; cols 1600q..1600q+1600
                        s0 = 1600 * q
                        h4r = spw.tile([8, 256], bf16, tag="h4r", name="h4r")
                        nc.sync.dma_start(
                            h4r[:, 0:200],
                            _apf(hcat[96:97, s0:s0 + 1],
                                 [[200, 8], [1, 200]]))
                        nc.vector.memset(h4r[:, 200:256], -1e30)
                        vals = spw.tile([8, 32], bf16, tag="vals",
                                        name="vals")
                        idxu = spw.tile([8, 32], mybir.dt.uint16, tag="idxu",
                                        name="idxu")
                        for r in range(4):
                            nc.vector.max(vals[:, 8 * r:8 * r + 8], h4r[:])
                            nc.vector.max_index(idxu[:, 8 * r:8 * r + 8],
                                                vals[:, 8 * r:8 * r + 8],
                                                h4r[:])
                            nc.vector.match_replace(h4r[:],
                                                    vals[:, 8 * r:8 * r + 8],
                                                    h4r[:], -1e30)
                        goff = spw.tile([8, 32], mybir.dt.uint16, tag="goff",
                                        name="goff")
                        nc.gpsimd.iota(goff[:], pattern=[[0, 32]], base=0,
                                       channel_multiplier=200)
                        nc.vector.tensor_tensor(out=idxu[:], in0=idxu[:],
                                                in1=goff[:], op=OP.add)
                        nc.sync.dma_start(
                            idxbuf_d.ap()[256 * q:256 * q + 256]
                            .rearrange("(g k) -> g k", g=8),
                            idxu[:].bitcast(mybir.dt.int16))
                        idxw = spw.tile([112, 16], mybir.dt.int16, tag="idxw",
                                        name="idxw")
                        nc.sync.dma_start(
                            idxw[0:16, :],
                            idxbuf_d.ap()[256 * q:256 * q + 256]
                            .rearrange("(c p) -> p c", p=16))
                        for rep in range(1, 7):
                            nc.sync.dma_start(idxw[16 * rep:16 * rep + 16, :],
                                              idxw[0:16, :])
                        nc.gpsimd.ap_gather(topsT[:, 256 * q:256 * q + 256],
                                            hcat32[:, s0:s0 + 1600],
                                            idxw[:], channels=112,
                                            num_elems=1600, d=1,
                                            num_idxs=256)
                        nc.gpsimd.tensor_copy(topsB[:, 256 * q:256 * q + 256],
                                               topsT[:, 256 * q:256 * q + 256])

                    # prologue: deg (32 rows) chunks -> rsqrt -> replicate
                    def deg_chunk(q):
                        c0, c1 = CW * q, CW * (q + 1)
                        dstage = stp.tile([32, CW], bf16, tag="dstage",
                                          name="dstage")
                        nc.scalar.dma_start(dstage[:], deg32_d.ap()[:, c0:c1])
                        nc.scalar.activation(dinv_rep[0:32, c0:c1],
                                             dstage[:],
                                             AF.Abs_reciprocal_sqrt)
                        for rep in range(1, 4):
                            nc.sync.dma_start(
                                dinv_rep[32 * rep:32 * rep + 32, c0:c1],
                                dinv_rep[0:32, c0:c1])

                    xstash = {}

                    def x_chunk(h):
                        xs = xsp.tile([128, 800], bf16, tag="xs", name="xs")
                        nc.gpsimd.dma_start(
                            xs[:], xT_d.ap()[:, 800 * h:800 * (h + 1)])
                        xstash[h] = xs

                    tiles = {0: stage_chunk(0), 1: stage_chunk(1)}
                    x_chunk(0)
                    x_chunk(1)
                    deg_chunk(0)
                    deg_chunk(1)
                    scale_chunk(0, tiles.pop(0))

                    for q in range(8):
                        if q + 2 < 8:
                            tiles[q + 2] = stage_chunk(q + 2)
                            deg_chunk(q + 2)
                        if q + 1 < 8:
                            scale_chunk(q + 1, tiles.pop(q + 1))
                        if 2 * q + 2 < 16:
                            x_chunk(2 * q + 2)
                        if 2 * q + 3 < 16:
                            x_chunk(2 * q + 3)
                        for l in range(4):
                            for r in range(4):
                                pair_layer(4 * q + r, l)
                                if l == 3:
                                    gp = 4 * q + r
                                    nc.vector.tensor_copy(
                                        hcat32[:, 400 * gp:400 * gp + 400],
                                        hcat[:, 400 * gp:400 * gp + 400])
                        sortpool_group(q)
                        if q == 1:
                            load_head_consts()
                        xstash.pop(2 * q, None)
                        xstash.pop(2 * q + 1, None)

            # ---- head: conv1(97->16) -> maxpool2 -> conv2(16->32,k=5)
            #      -> fc 352->128 -> fc 128->1 ----
            with (
                tc.tile_pool(name="head", bufs=2) as hp,
                tc.tile_pool(name="psH", bufs=1, space="PSUM") as psH,
            ):
                for q in range(4):
                    pc1 = psH.tile([16, 480], fp32, tag="c1", bufs=2,
                                   name="pc1")
                    rhs = _apf(topsB[0:97, 512 * q:512 * q + 1],
                               [[32, 16], [1, 30]])
                    nc.tensor.matmul(pc1[:], lhsT=cw1[:], rhs=rhs,
                                     start=True, stop=True)
                    nc.scalar.activation(c1T[:, 480 * q:480 * q + 480],
                                         pc1[:], AF.Relu, bias=cb1[:])
                poolT = hp.tile([16, 15 * GPC], bf16, tag="poolT")
                nc.vector.tensor_tensor(
                    out=_apf(poolT[0:16, 0:1], [[15, GPC], [1, 15]]),
                    in0=_apf(c1T[0:16, 0:1], [[30, GPC], [2, 15]]),
                    in1=_apf(c1T[0:16, 1:2], [[30, GPC], [2, 15]]),
                    op=OP.max)
                c2Te = hp.tile([33, 11 * GPC], bf16, tag="c2T")
                nc.vector.memset(c2Te[32:33, :], 1.0)
                for q in range(2):
                    pc2 = psH.tile([32, 352], fp32, tag="c2", bufs=2,
                                   name="pc2")
                    for t in range(5):
                        rhs = _apf(poolT[0:16, 480 * q + t:480 * q + t + 1],
                                   [[15, 32], [1, 11]])
                        nc.tensor.matmul(pc2[:], lhsT=cw2[t][:], rhs=rhs,
                                         start=(t == 0), stop=(t == 4))
                    nc.scalar.activation(c2Te[0:32, 352 * q:352 * q + 352],
                                         pc2[:], AF.Relu, bias=cb2[:])
                # fc1 graph-major: hlin[g, o2] via 11 accumulating matmuls
                # over conv positions; bias rides the ones row of c2Te.
                ph = psH.tile([64, 128], fp32, tag="hl")
                for p in range(11):
                    nc.tensor.matmul(ph[:],
                                     lhsT=_apf(c2Te[0:33, p:p + 1],
                                               [[11, GPC]]),
                                     rhs=lw1p[:, 128 * p:128 * p + 128],
                                     start=(p == 0), stop=(p == 10))
                hlin = hp.tile([64, 128], bf16, tag="hlin")
                nc.scalar.activation(hlin[:], ph[:], AF.Relu)
                pt = psH.tile([128, 64], bf16, tag="pT", name="pt")
                nc.tensor.transpose(pt[:], hlin[:], identb[:])
                hlinT = hp.tile([128, 64], bf16, tag="hlinT")
                nc.vector.tensor_copy(hlinT[:], pt[:])
                po = psH.tile([1, 64], fp32, tag="po")
                nc.tensor.matmul(po[:], lhsT=lw2[:], rhs=hlinT[:],
                                 start=True, stop=True)
                # sigmoid(z+b) = 0.5 + 0.5*tanh(0.5*(z+b)); reuses the tanh
                # table already loaded, skipping a 1.3us ACT_TABLE_LOAD
                lb2h = hp.tile([1, 1], fp32, tag="lb2h")
                nc.vector.tensor_scalar(out=lb2h[:], in0=lb2[:],
                                        scalar1=0.5, scalar2=None,
                                        op0=OP.mult)
                outT = hp.tile([1, 64], fp32, tag="outT")
                nc.scalar.activation(outT[:], po[:], AF.Tanh,
                                     bias=lb2h[:], scale=0.5)
                outS = hp.tile([1, 64], fp32, tag="outS")
                nc.vector.tensor_scalar(out=outS[:], in0=outT[:],
                                        scalar1=0.5, scalar2=0.5,
                                        op0=OP.mult, op1=OP.add)
                nc.sync.dma_start(out_d.ap(), outS[:])

    nc.compile()
    return nc


def _lw1p(lw1, lb1):
    """[352,128] fc1 weight -> [33, 11*128]: lw1p[o, 128p+o2] =
    lw1[11o+p, o2]; row 32 carries the bias (paired with the ones row of
    c2Te, emitted only in the p=0 slice so it is added exactly once)."""
    out = np.zeros((33, 11 * 128), np.float32)
    r = lw1.reshape(32, 11, 128)
    for p in range(11):
        out[0:32, 128 * p:128 * (p + 1)] = r[:, p, :]
    out[32, 0:128] = lb1
    return out


def _prep_inputs(inputs):
    """Shard + densify on host. Returns per-core in_maps."""
    import ml_dtypes
    bf = ml_dtypes.bfloat16
    x = np.asarray(inputs["x"], np.float32)
    ei = np.asarray(inputs["edge_index"], np.int64)
    src, dst = ei[0], ei[1]
    g_edge = dst // M
    jl = src - g_edge * M
    il = dst - g_edge * M
    flat = g_edge * (M * M) + jl * M + il
    cnt = np.bincount(flat, minlength=B * M * M).astype(np.float32)
    adj = cnt.reshape(B, M, M)
    adj += np.eye(M, dtype=np.float32)[None]
    deg = adj.sum(axis=1)                      # [B, M] integer-valued

    w234 = np.concatenate(
        [np.asarray(inputs["W2"], np.float32),
         np.asarray(inputs["W3"], np.float32),
         np.pad(np.asarray(inputs["W4"], np.float32), ((0, 0), (0, 31)))],
        axis=0)  # [96, 32]
    b4p = np.pad(np.asarray(inputs["b4"], np.float32), (0, 31))
    bgcn = np.stack(
        [np.asarray(inputs["b1"], np.float32),
         np.asarray(inputs["b2"], np.float32),
         np.asarray(inputs["b3"], np.float32), b4p], axis=1)  # [32, 4]
    cw1 = np.ascontiguousarray(
        np.asarray(inputs["convW1"], np.float32)[:, 0, :].T)  # [97,16]
    cw2_r = np.asarray(inputs["convW2"], np.float32)  # [32,16,5]
    cw2 = np.ascontiguousarray(
        cw2_r.transpose(2, 1, 0).reshape(80, 32))  # [(t,i),o]
    common = {
        "w1": np.asarray(inputs["W1"], np.float32).astype(bf),
        "w234": np.ascontiguousarray(w234).astype(bf),
        "bgcn": np.ascontiguousarray(bgcn),
        "cw1": cw1.astype(bf),
        "cb1": np.asarray(inputs["convb1"], np.float32).reshape(16, 1),
        "cw2": cw2.astype(bf),
        "cb2": np.asarray(inputs["convb2"], np.float32).reshape(32, 1),
        "lw1p": _lw1p(np.asarray(inputs["linW1"], np.float32),
                      np.asarray(inputs["linb1"], np.float32)).astype(bf),
        "lw2": np.asarray(inputs["linW2"], np.float32).astype(bf),
        "lb2": np.asarray(inputs["linb2"], np.float32).reshape(1, 1),
    }
    in_maps = []
    for c in range(NCORES):
        m = dict(common)
        xc = x[NPC * c:NPC * (c + 1)]
        m["xT"] = np.ascontiguousarray(xc.T).astype(bf)
        adjc = adj[GPC * c:GPC * (c + 1)]            # [64, 200, 200]
        m["adjlo"] = np.ascontiguousarray(
            adjc[:, 0:128, :].transpose(1, 0, 2).reshape(128, NPC)).astype(bf)
        m["adjhi"] = np.ascontiguousarray(
            adjc[:, 128:200, :].transpose(1, 0, 2).reshape(72, NPC)).astype(bf)
        degc = deg[GPC * c:GPC * (c + 1)]            # [64, 200]
        m["deg32"] = np.ascontiguousarray(
            np.broadcast_to(degc.reshape(1, NPC), (32, NPC))).astype(bf)
        dcol = np.ones((128, 128), np.float32)
        dcol[:, 0::2] = degc[:, 0:128].T             # col 2g <- nodes 0:128
        dcol[0:72, 1::2] = degc[:, 128:200].T        # col 2g+1 <- nodes hi
        m["degcol"] = dcol.astype(bf)
        in_maps.append(m)
    return in_maps


def _run(inputs, trace=False):
    from concourse import bass_utils
    if "nc" not in _STATE:
        _STATE["nc"] = _build()
    nc = _STATE["nc"]
    in_maps = _prep_inputs(inputs)
    res = bass_utils.run_bass_kernel_spmd(
        nc, in_maps, core_ids=list(range(NCORES)), trace=trace)
    out = np.concatenate([res.results[c]["out"].reshape(GPC)
                          for c in range(NCORES)])
    return out.reshape(B, 1).astype(np.float32), res


def kernel(**inputs) -> np.ndarray:
    out, _ = _run(inputs, trace=False)
    return out



# revision 45
# speedup vs baseline: 1.1152x; 1.0025x over previous
"""DGCNN (4x GCNConv + SortPool + Conv1d head) on 8 Trainium2 NeuronCores.

Data-parallel over graphs: each core owns 64 of the 512 graphs.
Per graph the GCN aggregation is computed densely in bf16:
    agg^T[f, i] = sum_j (y[j, f] * dinv[j]) * (adj[j, i] * dinv[i])
with adj the src-major dense adjacency-with-self-loops count matrix,
densified on the host from edge_index (a re-layout of the integer graph
structure); integer degree counts are likewise host re-layouts. All
floating-point math (rsqrt normalization, 4 GCN layers, SortPool top-k
selection+gather, conv/MLP head) runs on-device. Matmul operands are
bf16 (exact for the integer-valued adjacency), accumulation is fp32.

Perf notes vs the earlier version (262us -> ~225us):
- deg is shipped [32, NPC] and replicated to 128 partitions with on-chip
  SBUF DMAs instead of a 128-row HBM broadcast (saves 2.5MB of HBM).
- the [16, NPC] hcat memset (10.7us DVE prologue stall) is gone; the
  garbage rows 97:112 are gathered but never consumed.
- head-only weights DMA late so the prologue sync queue stages
  adjacency first; x arrives in [128, 800] chunks (fewer DMAs).
- SortPool h4 row is pulled via a direct SBUF->SBUF DMA (no DRAM trip).
- fc1 is computed graph-major as 11 accumulating matmuls over conv
  positions (weights pre-permuted host-side into lw1p, bias folded via
  a ones row), replacing 14 serial PE-transpose + DVE-copy pairs.
- the final sigmoid reuses the resident tanh table
  (sigmoid(x) = 0.5 + 0.5*tanh(x/2)), skipping an ACT_TABLE_LOAD.
"""

import numpy as np

B = 512
M = 200
GPC = 64            # graphs per core
NPC = GPC * M       # nodes per core
NCORES = 8
K = 30
F = 97

_STATE = {}


def _apf(base, pairs):
    """AP with the partition dim of `base` and custom free [step,count] pairs."""
    import concourse.bass as bass
    return bass.AP(tensor=base.tensor, offset=base.offset,
                   ap=[list(base.ap[0])] + [list(p) for p in pairs])


def _build(skip=()):
    skip = set(skip)
    import concourse.bass as bass
    import concourse.bacc as bacc
    import concourse.mybir as mybir
    from concourse.tile import TileContext
    from concourse.masks import make_identity

    fp32 = mybir.dt.float32
    bf16 = mybir.dt.bfloat16
    AF = mybir.ActivationFunctionType
    OP = mybir.AluOpType

    nc = bacc.Bacc("TRN2", target_bir_lowering=False, debug=False,
                   num_devices=NCORES)

    xT_d = nc.dram_tensor("xT", [128, NPC], bf16, kind="ExternalInput")
    adjlo_d = nc.dram_tensor("adjlo", [128, NPC], bf16, kind="ExternalInput")
    adjhi_d = nc.dram_tensor("adjhi", [72, NPC], bf16, kind="ExternalInput")
    deg32_d = nc.dram_tensor("deg32", [32, NPC], bf16, kind="ExternalInput")
    degcol_d = nc.dram_tensor("degcol", [128, 128], bf16, kind="ExternalInput")
    w1_d = nc.dram_tensor("w1", [128, 32], bf16, kind="ExternalInput")
    w234_d = nc.dram_tensor("w234", [96, 32], bf16, kind="ExternalInput")
    bgcn_d = nc.dram_tensor("bgcn", [32, 4], fp32, kind="ExternalInput")
    cw1_d = nc.dram_tensor("cw1", [97, 16], bf16, kind="ExternalInput")
    cb1_d = nc.dram_tensor("cb1", [16, 1], fp32, kind="ExternalInput")
    cw2_d = nc.dram_tensor("cw2", [80, 32], bf16, kind="ExternalInput")
    cb2_d = nc.dram_tensor("cb2", [32, 1], fp32, kind="ExternalInput")
    lw1p_d = nc.dram_tensor("lw1p", [33, 1408], bf16, kind="ExternalInput")
    lw2_d = nc.dram_tensor("lw2", [128, 1], bf16, kind="ExternalInput")
    lb2_d = nc.dram_tensor("lb2", [1, 1], fp32, kind="ExternalInput")

    idxbuf_d = nc.dram_tensor("idxbuf", [GPC * 32], mybir.dt.int16,
                              kind="Internal")
    out_d = nc.dram_tensor("out", [1, GPC], fp32, kind="ExternalOutput")

    NCH = 8                 # prologue chunks
    CW = NPC // NCH         # 1600 cols per chunk

    with TileContext(nc) as tc:
        with tc.tile_pool(name="const", bufs=1) as cp:
            identb = cp.tile([64, 64], bf16)
            make_identity(nc, identb[:])
            w1 = cp.tile([128, 32], bf16)
            nc.sync.dma_start(w1[:], w1_d.ap())
            w234 = cp.tile([96, 32], bf16)
            nc.sync.dma_start(w234[:], w234_d.ap())
            bgcn = cp.tile([32, 4], fp32)
            nc.sync.dma_start(bgcn[:], bgcn_d.ap())
            cw1 = cp.tile([97, 16], bf16)
            cb1 = cp.tile([16, 1], fp32)
            cw2 = [cp.tile([16, 32], bf16, tag=f"cw2_{t}", name=f"cw2_{t}")
                   for t in range(5)]
            cb2 = cp.tile([32, 1], fp32)
            lw1p = cp.tile([33, 1408], bf16)
            lw2 = cp.tile([128, 1], bf16)
            lb2 = cp.tile([1, 1], fp32)

            def load_head_consts():
                # deferred: head weights aren't needed until the tail, so
                # keep their DMAs out of the prologue's sync queue
                nc.sync.dma_start(cw1[:], cw1_d.ap())
                nc.sync.dma_start(cb1[:], cb1_d.ap())
                for t in range(5):
                    nc.sync.dma_start(cw2[t][:],
                                      cw2_d.ap()[16 * t:16 * t + 16, :])
                nc.sync.dma_start(cb2[:], cb2_d.ap())
                nc.sync.dma_start(lw1p[:], lw1p_d.ap())
                nc.sync.dma_start(lw2[:], lw2_d.ap())
                nc.sync.dma_start(lb2[:], lb2_d.ap())

            # per-node 1/sqrt(deg) columns: col 2g = nodes 0:128 of graph g,
            # col 2g+1 = nodes 128:200 (on partitions 0:72)
            dinv_col = cp.tile([128, 128], fp32)
            dcol_s = cp.tile([128, 128], bf16, name="dcol_s")
            nc.sync.dma_start(dcol_s[:], degcol_d.ap())
            nc.scalar.activation(dinv_col[:], dcol_s[:],
                                 AF.Abs_reciprocal_sqrt)

            # hcat rows: 0:32 h1, 32:64 h2, 64:96 h3, 96 h4; col = 256*g + i
            hcat = cp.tile([112, 200 * GPC], bf16)
            topsT = cp.tile([112, 32 * GPC], fp32)
            topsB = cp.tile([112, 32 * GPC], bf16)
            c1T = cp.tile([16, 30 * GPC], bf16)

            if "agg" in skip:
                nc.gpsimd.memset(hcat[:], 0.25)

            with tc.tile_pool(name="gcn", bufs=1) as gp_pool:
                adjS_lo = gp_pool.tile([128, NPC], bf16)
                adjS_hi = gp_pool.tile([72, NPC], bf16)
                dinv_rep = gp_pool.tile([128, NPC], bf16)
                hcat32 = gp_pool.tile([112, 200 * GPC], fp32)

                with (
                    tc.tile_pool(name="stage", bufs=3) as stp,
                    tc.tile_pool(name="work", bufs=4) as wp,
                    tc.tile_pool(name="spw", bufs=2) as spw,
                    tc.tile_pool(name="xsp", bufs=4) as xsp,
                    tc.tile_pool(name="psY", bufs=4, space="PSUM") as psY,
                    tc.tile_pool(name="psG", bufs=4, space="PSUM") as psG,
                ):
                    def stage_chunk(q):
                        c0, c1 = CW * q, CW * (q + 1)
                        astage = stp.tile([128, CW], bf16, tag="astage",
                                          name="astage")
                        nc.sync.dma_start(astage[:], adjlo_d.ap()[:, c0:c1])
                        hstage = stp.tile([72, CW], bf16, tag="hstage",
                                          name="hstage")
                        nc.scalar.dma_start(hstage[:], adjhi_d.ap()[:, c0:c1])
                        return astage, hstage

                    def scale_chunk(q, tiles):
                        c0, c1 = CW * q, CW * (q + 1)
                        astage, hstage = tiles
                        nc.vector.tensor_tensor(
                            out=adjS_lo[:, c0:c1], in0=astage[:],
                            in1=dinv_rep[:, c0:c1], op=OP.mult)
                        nc.vector.tensor_tensor(
                            out=adjS_hi[:, c0:c1], in0=hstage[:],
                            in1=dinv_rep[0:72, c0:c1], op=OP.mult)

                    def pair_layer(gp, l):
                        fo = 32 if l < 3 else 1
                        py = psY.tile([128, 128], fp32, tag="y", name="py")
                        y = wp.tile([128, 128], bf16, tag="y_s", name="y")
                        for half in range(2):
                            g = 2 * gp + half
                            yo = 64 * half
                            if l == 0:
                                xs = xstash[gp // 2]
                                xo = 400 * (gp % 2) + 200 * half
                                lhs_lo = xs[:, xo:xo + 128]
                                lhs_hi = xs[:, xo + 128:xo + 200]
                                w_t = w1[:, 0:fo]
                            else:
                                r0 = 32 * (l - 1)
                                c0 = 200 * g
                                lhs_lo = hcat[r0:r0 + 32, c0:c0 + 128]
                                lhs_hi = hcat[r0:r0 + 32, c0 + 128:c0 + 200]
                                w_t = w234[r0:r0 + 32, 0:fo]
                            nc.tensor.matmul(py[:, yo:yo + fo],
                                             lhsT=lhs_lo, rhs=w_t,
                                             start=True, stop=True)
                            nc.tensor.matmul(py[0:72, yo + 32:yo + 32 + fo],
                                             lhsT=lhs_hi, rhs=w_t,
                                             start=True, stop=True)
                        # drain xw, row scale dinv[j]; blocks {lo, hi} x 2
                        nc.vector.tensor_tensor(
                            out=_apf(y[0:128, 0:1], [[64, 2], [1, fo]]),
                            in0=_apf(py[0:128, 0:1], [[64, 2], [1, fo]]),
                            in1=_apf(dinv_col[0:128, 4 * gp:4 * gp + 1],
                                     [[2, 2], [0, fo]]),
                            op=OP.mult)
                        nc.vector.tensor_tensor(
                            out=_apf(y[0:72, 32:33], [[64, 2], [1, fo]]),
                            in0=_apf(py[0:72, 32:33], [[64, 2], [1, fo]]),
                            in1=_apf(dinv_col[0:72,
                                     4 * gp + 1:4 * gp + 2],
                                     [[2, 2], [0, fo]]),
                            op=OP.mult)
                        pagg = psG.tile([32, 456], fp32, tag="agg",
                                        name="pagg")
                        for half in range(2):
                            g = 2 * gp + half
                            yo, co = 64 * half, 256 * half
                            nc.tensor.matmul(
                                pagg[0:fo, co:co + 200],
                                lhsT=y[:, yo:yo + fo],
                                rhs=adjS_lo[:, 200 * g:200 * g + 200],
                                start=True, stop=False)
                            nc.tensor.matmul(
                                pagg[0:fo, co:co + 200],
                                lhsT=y[0:72, yo + 32:yo + 32 + fo],
                                rhs=adjS_hi[:, 200 * g:200 * g + 200],
                                start=False, stop=True)
                        r0 = 32 * l if l < 3 else 96
                        nc.scalar.activation(
                            hcat[r0:r0 + fo, 400 * gp:400 * gp + 400],
                            _apf(pagg[0:fo, 0:1], [[256, 2], [1, 200]]),
                            AF.Tanh, bias=bgcn[0:fo, l:l + 1])

                    def sortpool_group(q):
                        # top-30 for graphs 8q..8q+8; cols 1600q..1600q+1600
                        s0 = 1600 * q
                        h4r = spw.tile([8, 256], bf16, tag="h4r", name="h4r")
                        nc.sync.dma_start(
                            h4r[:, 0:200],
                            _apf(hcat[96:97, s0:s0 + 1],
                                 [[200, 8], [1, 200]]))
                        nc.vector.memset(h4r[:, 200:256], -1e30)
                        vals = spw.tile([8, 32], bf16, tag="vals",
                                        name="vals")
                        idxu = spw.tile([8, 32], mybir.dt.uint16, tag="idxu",
                                        name="idxu")
                        for r in range(4):
                            nc.vector.max(vals[:, 8 * r:8 * r + 8], h4r[:])
                            nc.vector.max_index(idxu[:, 8 * r:8 * r + 8],
                                                vals[:, 8 * r:8 * r + 8],
                                                h4r[:])
                            nc.vector.match_replace(h4r[:],
                                                    vals[:, 8 * r:8 * r + 8],
                                                    h4r[:], -1e30)
                        goff = spw.tile([8, 32], mybir.dt.uint16, tag="goff",
                                        name="goff")
                        nc.gpsimd.iota(goff[:], pattern=[[0, 32]], base=0,
                                       channel_multiplier=200)
                        nc.vector.tensor_tensor(out=idxu[:], in0=idxu[:],
                                                in1=goff[:], op=OP.add)
                        nc.sync.dma_start(
                            idxbuf_d.ap()[256 * q:256 * q + 256]
                            .rearrange("(g k) -> g k", g=8),
                            idxu[:].bitcast(mybir.dt.int16))
                        idxw = spw.tile([112, 16], mybir.dt.int16, tag="idxw",
                                        name="idxw")
                        nc.sync.dma_start(
                            idxw[0:16, :],
                            idxbuf_d.ap()[256 * q:256 * q + 256]
                            .rearrange("(c p) -> p c", p=16))
                        for rep in range(1, 7):
                            nc.sync.dma_start(idxw[16 * rep:16 * rep + 16, :],
                                              idxw[0:16, :])
                        nc.gpsimd.ap_gather(topsT[:, 256 * q:256 * q + 256],
                                            hcat32[:, s0:s0 + 1600],
                                            idxw[:], channels=112,
                                            num_elems=1600, d=1,
                                            num_idxs=256)
                        nc.gpsimd.tensor_copy(topsB[:, 256 * q:256 * q + 256],
                                               topsT[:, 256 * q:256 * q + 256])

                    # prologue: deg (32 rows) chunks -> rsqrt -> replicate
                    def deg_chunk(q):
                        c0, c1 = CW * q, CW * (q + 1)
                        dstage = stp.tile([32, CW], bf16, tag="dstage",
                                          name="dstage")
                        nc.scalar.dma_start(dstage[:], deg32_d.ap()[:, c0:c1])
                        nc.scalar.activation(dinv_rep[0:32, c0:c1],
                                             dstage[:],
                                             AF.Abs_reciprocal_sqrt)
                        for rep in range(1, 4):
                            nc.sync.dma_start(
                                dinv_rep[32 * rep:32 * rep + 32, c0:c1],
                                dinv_rep[0:32, c0:c1])

                    xstash = {}

                    def x_chunk(h):
                        xs = xsp.tile([128, 800], bf16, tag="xs", name="xs")
                        nc.gpsimd.dma_start(
                            xs[:], xT_d.ap()[:, 800 * h:800 * (h + 1)])
                        xstash[h] = xs

                    tiles = {0: stage_chunk(0), 1: stage_chunk(1)}
                    x_chunk(0)
                    x_chunk(1)
                    deg_chunk(0)
                    deg_chunk(1)
                    scale_chunk(0, tiles.pop(0))

                    for q in range(8):
                        if q + 2 < 8:
                            tiles[q + 2] = stage_chunk(q + 2)
                            deg_chunk(q + 2)
                        if q + 1 < 8:
                            scale_chunk(q + 1, tiles.pop(q + 1))
                        if 2 * q + 2 < 16:
                            x_chunk(2 * q + 2)
                        if 2 * q + 3 < 16:
                            x_chunk(2 * q + 3)
                        for l in range(4):
                            for r in range(4):
                                pair_layer(4 * q + r, l)
                                if l == 3:
                                    gp = 4 * q + r
                                    nc.vector.tensor_copy(
                                        hcat32[:, 400 * gp:400 * gp + 400],
                                        hcat[:, 400 * gp:400 * gp + 400])
                        sortpool_group(q)
                        if q == 1:
                            load_head_consts()
                        xstash.pop(2 * q, None)
                        xstash.pop(2 * q + 1, None)

            # ---- head: conv1(97->16) -> maxpool2 -> conv2(16->32,k=5)
            #      -> fc 352->128 -> fc 128->1 ----
            with (
                tc.tile_pool(name="head", bufs=2) as hp,
                tc.tile_pool(name="psH", bufs=1, space="PSUM") as psH,
            ):
                for q in range(4):
                    pc1 = psH.tile([16, 480], fp32, tag="c1", bufs=2,
                                   name="pc1")
                    rhs = _apf(topsB[0:97, 512 * q:512 * q + 1],
                               [[32, 16], [1, 30]])
                    nc.tensor.matmul(pc1[:], lhsT=cw1[:], rhs=rhs,
                                     start=True, stop=True)
                    nc.scalar.activation(c1T[:, 480 * q:480 * q + 480],
                                         pc1[:], AF.Relu, bias=cb1[:])
                poolT = hp.tile([16, 15 * GPC], bf16, tag="poolT")
                nc.vector.tensor_tensor(
                    out=_apf(poolT[0:16, 0:1], [[15, GPC], [1, 15]]),
                    in0=_apf(c1T[0:16, 0:1], [[30, GPC], [2, 15]]),
                    in1=_apf(c1T[0:16, 1:2], [[30, GPC], [2, 15]]),
                    op=OP.max)
                c2Te = hp.tile([33, 11 * GPC], bf16, tag="c2T")
                nc.vector.memset(c2Te[32:33, :], 1.0)
                for q in range(2):
                    pc2 = psH.tile([32, 352], fp32, tag="c2", bufs=2,
                                   name="pc2")
                    for t in range(5):
                        rhs = _apf(poolT[0:16, 480 * q + t:480 * q + t + 1],
                                   [[15, 32], [1, 11]])
                        nc.tensor.matmul(pc2[:], lhsT=cw2[t][:], rhs=rhs,
                                         start=(t == 0), stop=(t == 4))
                    nc.scalar.activation(c2Te[0:32, 352 * q:352 * q + 352],
                                         pc2[:], AF.Relu, bias=cb2[:])
                # fc1 graph-major: hlin[g, o2] via 11 accumulating matmuls
                # over conv positions; bias rides the ones row of c2Te.
                ph = psH.tile([64, 128], fp32, tag="hl")
                for p in range(11):
                    nc.tensor.matmul(ph[:],
                                     lhsT=_apf(c2Te[0:33, p:p + 1],
                                               [[11, GPC]]),
                                     rhs=lw1p[:, 128 * p:128 * p + 128],
                                     start=(p == 0), stop=(p == 10))
                hlin = hp.tile([64, 128], bf16, tag="hlin")
                nc.scalar.activation(hlin[:], ph[:], AF.Relu)
                pt = psH.tile([128, 64], bf16, tag="pT", name="pt")
                nc.tensor.transpose(pt[:], hlin[:], identb[:])
                hlinT = hp.tile([128, 64], bf16, tag="hlinT")
                nc.vector.tensor_copy(hlinT[:], pt[:])
                po = psH.tile([1, 64], fp32, tag="po")
                nc.tensor.matmul(po[:], lhsT=lw2[:], rhs=hlinT[:],
                                 start=True, stop=True)
                # sigmoid(z+b) = 0.5 + 0.5*tanh(0.5*(z+b)); reuses the tanh
                # table already loaded, skipping a 1.3us ACT_TABLE_LOAD
                lb2h = hp.tile([1, 1], fp32, tag="lb2h")
                nc.vector.tensor_scalar(out=lb2h[:], in0=lb2[:],
                                        scalar1=0.5, scalar2=None,
                                        op0=OP.mult)
                outT = hp.tile([1, 64], fp32, tag="outT")
                nc.scalar.activation(outT[:], po[:], AF.Tanh,
                                     bias=lb2h[:], scale=0.5)
                outS = hp.tile([1, 64], fp32, tag="outS")
                nc.vector.tensor_scalar(out=outS[:], in0=outT[:],
                                        scalar1=0.5, scalar2=0.5,
                                        op0=OP.mult, op1=OP.add)
                nc.sync.dma_start(out_d.ap(), outS[:])

    nc.compile()
    return nc


def _lw1p(lw1, lb1):
    """[352,128] fc1 weight -> [33, 11*128]: lw1p[o, 128p+o2] =
    lw1[11o+p, o2]; row 32 carries the bias (paired with the ones row of
    c2Te, emitted only in the p=0 slice so it is added exactly once)."""
    out = np.zeros((33, 11 * 128), np.float32)
    r = lw1.reshape(32, 11, 128)
    for p in range(11):
        out[0:32, 128 * p:128 * (p + 1)] = r[:, p, :]
    out[32, 0:128] = lb1
    return out


def _prep_inputs(inputs):
    """Shard + densify on host. Returns per-core in_maps."""
    import ml_dtypes
    bf = ml_dtypes.bfloat16
    x = np.asarray(inputs["x"], np.float32)
    ei = np.asarray(inputs["edge_index"], np.int64)
    src, dst = ei[0], ei[1]
    g_edge = dst // M
    jl = src - g_edge * M
    il = dst - g_edge * M
    flat = g_edge * (M * M) + jl * M + il
    cnt = np.bincount(flat, minlength=B * M * M).astype(np.float32)
    adj = cnt.reshape(B, M, M)
    adj += np.eye(M, dtype=np.float32)[None]
    deg = adj.sum(axis=1)                      # [B, M] integer-valued

    w234 = np.concatenate(
        [np.asarray(inputs["W2"], np.float32),
         np.asarray(inputs["W3"], np.float32),
         np.pad(np.asarray(inputs["W4"], np.float32), ((0, 0), (0, 31)))],
        axis=0)  # [96, 32]
    b4p = np.pad(np.asarray(inputs["b4"], np.float32), (0, 31))
    bgcn = np.stack(
        [np.asarray(inputs["b1"], np.float32),
         np.asarray(inputs["b2"], np.float32),
         np.asarray(inputs["b3"], np.float32), b4p], axis=1)  # [32, 4]
    cw1 = np.ascontiguousarray(
        np.asarray(inputs["convW1"], np.float32)[:, 0, :].T)  # [97,16]
    cw2_r = np.asarray(inputs["convW2"], np.float32)  # [32,16,5]
    cw2 = np.ascontiguousarray(
        cw2_r.transpose(2, 1, 0).reshape(80, 32))  # [(t,i),o]
    common = {
        "w1": np.asarray(inputs["W1"], np.float32).astype(bf),
        "w234": np.ascontiguousarray(w234).astype(bf),
        "bgcn": np.ascontiguousarray(bgcn),
        "cw1": cw1.astype(bf),
        "cb1": np.asarray(inputs["convb1"], np.float32).reshape(16, 1),
        "cw2": cw2.astype(bf),
        "cb2": np.asarray(inputs["convb2"], np.float32).reshape(32, 1),
        "lw1p": _lw1p(np.asarray(inputs["linW1"], np.float32),
                      np.asarray(inputs["linb1"], np.float32)).astype(bf),
        "lw2": np.asarray(inputs["linW2"], np.float32).astype(bf),
        "lb2": np.asarray(inputs["linb2"], np.float32).reshape(1, 1),
    }
    in_maps = []
    for c in range(NCORES):
        m = dict(common)
        xc = x[NPC * c:NPC * (c + 1)]
        m["xT"] = np.ascontiguousarray(xc.T).astype(bf)
        adjc = adj[GPC * c:GPC * (c + 1)]            # [64, 200, 200]
        m["adjlo"] = np.ascontiguousarray(
            adjc[:, 0:128, :].transpose(1, 0, 2).reshape(128, NPC)).astype(bf)
        m["adjhi"] = np.ascontiguousarray(
            adjc[:, 128:200, :].transpose(1, 0, 2).reshape(72, NPC)).astype(bf)
        degc = deg[GPC * c:GPC * (c + 1)]            # [64, 200]
        m["deg32"] = np.ascontiguousarray(
            np.broadcast_to(degc.reshape(1, NPC), (32, NPC))).astype(bf)
        dcol = np.ones((128, 128), np.float32)
        dcol[:, 0::2] = degc[:, 0:128].T             # col 2g <- nodes 0:128
        dcol[0:72, 1::2] = degc[:, 128:200].T        # col 2g+1 <- nodes hi
        m["degcol"] = dcol.astype(bf)
        in_maps.append(m)
    return in_maps


def _run(inputs, trace=False):
    from concourse import bass_utils
    if "nc" not in _STATE:
        _STATE["nc"] = _build()
    nc = _STATE["nc"]
    in_maps = _prep_inputs(inputs)
    res = bass_utils.run_bass_kernel_spmd(
        nc, in_maps, core_ids=list(range(NCORES)), trace=trace)
    out = np.concatenate([res.results[c]["out"].reshape(GPC)
                          for c in range(NCORES)])
    return out.reshape(B, 1).astype(np.float32), res


def kernel(**inputs) -> np.ndarray:
    out, _ = _run(inputs, trace=False)
    return out

